# revision 7
# baseline (speedup 1.0000x reference)
"""Trainium2 Bass kernel: 14-qubit data-reuploading quantum circuit actor.

Circuit per layer l (NL=5):
  for w in 0..13:  RY(in_scale[l,w]*x[:,w]) ; RZ(in_scale[l,w+14]*x[:,w]) on wire w
  for w in 0..13:  RZ(weights[l,w]) on wire w          (merged into input RZ)
  for w in 0..13:  RY(weights[l,w+14]) on wire w
  CNOT ring (i -> i+1 mod 14)
Then <Z_w> for w in 0..5, * action_scale + action_bias.

This environment's cost model (measured): per-instruction ~30-100us,
per-element ~0, per-call fixed ~200ms.  So the kernel minimizes
INSTRUCTION COUNT:

  - state: ONE fp16 SBUF plane per buffer, complex-interleaved
    [128 batch-partitions, 32768 floats] (float f = 2*amp + (0=re,1=im)),
    double-buffered A<->B: every gate reads A, writes B, swap.  No
    copy-backs, no temp shuffling, no deferred cosines (exact rotations
    keep |amp|<=1, fp16-safe).
  - custom DVE op ROT2: out = s0*in0 + s1*in1 (two per-partition scalars)
    -> a full RY gate (both complex comps, both halves) = 2 instructions;
    RZ = 4 (per-half phase sign differs); RY_weight with ring-CNOT folded
    into write APs = 4 (2 for wire 0).
  - CNOT(13,0) folded into next layer's RY(0) read APs; for the last
    layer folded into the sqsum (measurement) read APs.
  - measurement: SQSUM custom op (re^2+im^2) -> 64 block sums on chip;
    +/- signs applied on host.
  ~707 instructions per 128-row tile, 2 tiles per core (batch 2048 over
  8 cores = 256 rows/core).
"""

import os
import numpy as np

NQ = 14
NL = 5
OBS = 14
NA = 6
B = 2048
NCORES = 8
BPC = B // NCORES          # 256 batch rows per core
PT = 128                   # partitions (batch rows) per tile
NTILES = BPC // PT         # 2
NS = 1 << NQ               # 16384 amplitudes
F = 2 * NS                 # 32768 floats per row (complex-interleaved)
NCPG = 9                   # coef columns per (layer, wire)
NCOLS = NL * NQ * NCPG     # 630
# col offsets within a (l,w) group
RY_C, RY_S, RY_NS = 0, 1, 2
RZ_C, RZ_S, RZ_NS = 3, 4, 5
WY_C, WY_S, WY_NS = 6, 7, 8

# ---------------------------------------------------------------- host tables


def col(l, w, k):
    return (l * NQ + w) * NCPG + k


def coef_table(x, input_scaling, weights):
    """(n, NCOLS) f32 of cos/sin half-angle coefficients per (l, w)."""
    x = np.asarray(x, np.float64)
    isc = np.asarray(input_scaling, np.float64)
    wt = np.asarray(weights, np.float64)
    n = x.shape[0]
    tab = np.zeros((n, NL, NQ, NCPG), np.float64)
    for l in range(NL):
        for w in range(NQ):
            a_ry = isc[l, w] * x[:, w] / 2.0
            a_rz = (isc[l, w + OBS] * x[:, w] + wt[l, w]) / 2.0
            a_wy = np.full(n, wt[l, w + NQ] / 2.0)
            for base, a in ((RY_C, a_ry), (RZ_C, a_rz), (WY_C, a_wy)):
                tab[:, l, w, base] = np.cos(a)
                tab[:, l, w, base + 1] = np.sin(a)
                tab[:, l, w, base + 2] = -np.sin(a)
    return tab.reshape(n, NCOLS).astype(np.float32)


def postprocess(s64, action_scale, action_bias):
    """s64: (n, 64) block sums (blocks = top-6 amp bits). -> (n, NA) f32."""
    blk = np.arange(64)
    out = np.zeros((s64.shape[0], NA), np.float32)
    for w in range(NA):
        sign = 1.0 - 2.0 * ((blk >> (5 - w)) & 1)
        out[:, w] = s64 @ sign.astype(np.float32)
    return out * np.asarray(action_scale, np.float32) + np.asarray(
        action_bias, np.float32
    )


# ------------------------------------------------------------- gate schedule
# region = (buf, offset, dims); dims = tuple of (step, count), innermost
# last, in float-index space (f = 2*amp + comp).  Buffers: "A"/"B" full
# state planes, "T"/"U" 16384-float scratch (compacted wire halves; "T"
# doubles as the probability plane P at measurement), "S" the s64 output.
# ops:
#   ("rot2", dst, s0, s1, c0, c1): dst = c0*s0 + c1*s1.  The custom-DVE
#       TTSS encoding requires s1 to have a SINGLE free dim (rank-1).
#   ("cp", dst, src):              dst = src (staging copy)
#   ("sqsum", dst, s0, s1):        dst = s0^2 + s1^2 (rank-1 s1)
#   ("red", dst, src):             s64 block reduce


def _norm(dims):
    """drop unit dims, merge contiguous, assert rank<=2."""
    d = [(s, c) for s, c in dims if c != 1]
    out = []
    for s, c in d:
        if out and out[-1][0] == s * c:
            out[-1] = (s, c * out[-1][1])
        else:
            out.append((s, c))
    if not out:
        out = [(1, 1)]
    assert len(out) <= 2, out
    return tuple(out)


def _reg(buf, off, *dims):
    return (buf, off, _norm(dims))


class Sched:
    def __init__(self):
        self.ops = []
        self.cur = "A"

    def swap(self):
        self.cur = "B" if self.cur == "A" else "A"

    def rot2(self, dst, s0, s1, c0, c1):
        assert len(s1[2]) == 1, ("rot2 s1 must be rank-1", s1)
        self.ops.append(("rot2", dst, s0, s1, c0, c1))

    def cp(self, dst, src):
        self.ops.append(("cp", dst, src))

    # wire-0 gates: halves are contiguous -> rank-1, no staging ------------
    def ry0(self, cc, cs, cns):
        a, b = self.cur, "B" if self.cur == "A" else "A"
        X = _reg(a, 0, (1, NS))
        Y = _reg(a, NS, (1, NS))
        self.rot2(_reg(b, 0, (1, NS)), X, Y, cc, cns)
        self.rot2(_reg(b, NS, (1, NS)), X, Y, cs, cc)
        self.swap()

    def ry0_fold(self, cc, cs, cns):
        """RY(0) reading through the previous layer's CNOT(13, 0): stage the
        logical Y half into U piece-major (lsb0 | lsb1), then 4 rot2."""
        a, b = self.cur, "B" if self.cur == "A" else "A"
        d = ((4, 4096), (1, 2))
        E00, E01 = _reg(a, 0, *d), _reg(a, 2, *d)
        E10, E11 = _reg(a, 16384, *d), _reg(a, 16386, *d)
        U0, U1 = _reg("U", 0, (1, 8192)), _reg("U", 8192, (1, 8192))
        self.cp(U0, E10)  # logical Y, lsb=0
        self.cp(U1, E01)  # logical Y, lsb=1 (msb-flipped physically)
        B00, B01 = _reg(b, 0, *d), _reg(b, 2, *d)
        B10, B11 = _reg(b, 16384, *d), _reg(b, 16386, *d)
        self.rot2(B00, E00, U0, cc, cns)
        self.rot2(B10, E00, U0, cs, cc)
        self.rot2(B01, E11, U1, cc, cns)  # lsb=1: X lives msb-flipped
        self.rot2(B11, E11, U1, cs, cc)
        self.swap()

    def rz0(self, cc, cs, cns):
        a, b = self.cur, "B" if self.cur == "A" else "A"

        def r(buf, off):
            return _reg(buf, off, (2, 8192))

        self.rot2(r(b, 0), r(a, 0), r(a, 1), cc, cs)
        self.rot2(r(b, 1), r(a, 0), r(a, 1), cns, cc)
        self.rot2(r(b, NS), r(a, NS), r(a, NS + 1), cc, cns)
        self.rot2(r(b, NS + 1), r(a, NS), r(a, NS + 1), cs, cc)
        self.swap()

    # generic wires: RY into scratch T/U, RZ from scratch back to state ----
    def ry_rz(self, w, ryc, rys, ryns, rzc, rzs, rzns):
        a, b = self.cur, "B" if self.cur == "A" else "A"
        sa = 1 << (13 - w)
        nb = 1 << w
        X = _reg(a, 0, (4 * sa, nb), (1, 2 * sa))
        Y = _reg(a, 2 * sa, (4 * sa, nb), (1, 2 * sa))
        T = _reg("T", 0, (1, NS))
        U = _reg("U", 0, (1, NS))
        self.cp(U, Y)
        self.rot2(T, X, U, ryc, ryns)   # X' compacted
        self.rot2(U, X, U, rys, ryc)    # Y' in place
        Tre, Tim = _reg("T", 0, (2, 8192)), _reg("T", 1, (2, 8192))
        Ure, Uim = _reg("U", 0, (2, 8192)), _reg("U", 1, (2, 8192))
        Xre = _reg(b, 0, (4 * sa, nb), (2, sa))
        Xim = _reg(b, 1, (4 * sa, nb), (2, sa))
        Yre = _reg(b, 2 * sa, (4 * sa, nb), (2, sa))
        Yim = _reg(b, 2 * sa + 1, (4 * sa, nb), (2, sa))
        self.rot2(Xre, Tre, Tim, rzc, rzs)
        self.rot2(Xim, Tre, Tim, rzns, rzc)
        self.rot2(Yre, Ure, Uim, rzc, rzns)
        self.rot2(Yim, Ure, Uim, rzs, rzc)
        self.swap()

    def ry_fold_cnot(self, t, cc, cs, cns):
        """RY_w(t) with CNOT(t-1, t) folded into write APs (t >= 1)."""
        a, b = self.cur, "B" if self.cur == "A" else "A"
        fst = 2 * (1 << (13 - t))
        pt = 2 * fst
        P2 = 2 * pt
        nb = 1 << (t - 1)

        def R(buf, ai, bi):
            return _reg(buf, ai * pt + bi * fst, (P2, nb), (1, fst))

        if t == 1:  # nb == 1: all regions rank-1, no staging
            s01, s11 = R(a, 0, 1), R(a, 1, 1)
        else:
            s01, s11 = _reg("T", 0, (1, 8192)), _reg("T", 8192, (1, 8192))
            self.cp(s01, R(a, 0, 1))
            self.cp(s11, R(a, 1, 1))
        self.rot2(R(b, 0, 0), R(a, 0, 0), s01, cc, cns)
        self.rot2(R(b, 0, 1), R(a, 0, 0), s01, cs, cc)
        self.rot2(R(b, 1, 1), R(a, 1, 0), s11, cc, cns)  # X' -> flipped
        self.rot2(R(b, 1, 0), R(a, 1, 0), s11, cs, cc)   # Y' -> flipped
        self.swap()

    def measurement(self):
        a = self.cur
        # P[amp] = |state[pi(amp)]|^2, pi = last layer's CNOT(13,0) perm.
        # P lives in T.
        self.ops.append(
            ("sqsum", _reg("T", 0, (2, 8192)),
             _reg(a, 0, (4, 8192)), _reg(a, 1, (4, 8192)))
        )
        self.ops.append(
            ("sqsum", _reg("T", 1, (2, 4096)),
             _reg(a, 16386, (4, 4096)), _reg(a, 16387, (4, 4096)))
        )
        self.ops.append(
            ("sqsum", _reg("T", 8193, (2, 4096)),
             _reg(a, 2, (4, 4096)), _reg(a, 3, (4, 4096)))
        )
        self.ops.append(
            ("red", ("S", 0, ((1, 64),)), ("T", 0, ((256, 64), (1, 256))))
        )


def build_schedule():
    S = Sched()
    for l in range(NL):
        for w in range(NQ):
            if w == 0:
                if l == 0:
                    S.ry0(col(l, 0, RY_C), col(l, 0, RY_S), col(l, 0, RY_NS))
                else:
                    S.ry0_fold(col(l, 0, RY_C), col(l, 0, RY_S), col(l, 0, RY_NS))
                S.rz0(col(l, 0, RZ_C), col(l, 0, RZ_S), col(l, 0, RZ_NS))
            else:
                S.ry_rz(
                    w,
                    col(l, w, RY_C), col(l, w, RY_S), col(l, w, RY_NS),
                    col(l, w, RZ_C), col(l, w, RZ_S), col(l, w, RZ_NS),
                )
        S.ry0(col(l, 0, WY_C), col(l, 0, WY_S), col(l, 0, WY_NS))
        for t in range(1, NQ):
            S.ry_fold_cnot(t, col(l, t, WY_C), col(l, t, WY_S), col(l, t, WY_NS))
    S.measurement()
    return S.ops


# ------------------------------------------------------------ numpy executor


def _indices(reg):
    _, off, dims = reg
    idx = np.array([0], np.int64)
    for st, ct in dims:
        idx = (idx[:, None] + (np.arange(ct, dtype=np.int64) * st)[None, :]).ravel()
    return off + idx


def simulate_numpy(tab, fp16=True):
    """tab: (n, NCOLS) f32 coef table -> (n, 64) block sums."""
    n = tab.shape[0]
    sdt = np.float16 if fp16 else np.float32
    bufs = {
        "A": np.zeros((n, F), sdt),
        "B": np.zeros((n, F), sdt),
        "T": np.zeros((n, NS), sdt),
        "U": np.zeros((n, NS), sdt),
        "S": np.zeros((n, 64), np.float32),
    }
    bufs["A"][:, 0] = 1.0
    A = tab
    for op in build_schedule():
        kind = op[0]
        if kind == "rot2":
            _, dst, s0, s1, c0, c1 = op
            v = (
                A[:, c0 : c0 + 1].astype(np.float32)
                * bufs[s0[0]][:, _indices(s0)].astype(np.float32)
                + A[:, c1 : c1 + 1].astype(np.float32)
                * bufs[s1[0]][:, _indices(s1)].astype(np.float32)
            )
            bufs[dst[0]][:, _indices(dst)] = v.astype(sdt)
        elif kind == "cp":
            _, dst, src = op
            bufs[dst[0]][:, _indices(dst)] = bufs[src[0]][:, _indices(src)]
        elif kind == "sqsum":
            _, dst, s0, s1 = op
            v = (
                bufs[s0[0]][:, _indices(s0)].astype(np.float32) ** 2
                + bufs[s1[0]][:, _indices(s1)].astype(np.float32) ** 2
            )
            bufs[dst[0]][:, _indices(dst)] = v.astype(sdt)
        elif kind == "red":
            _, dst, src = op
            v = bufs[src[0]][:, _indices(src)].astype(np.float32)
            bufs["S"][:, _indices(dst)] = v.reshape(n, 64, 256).sum(axis=2)
        else:
            raise ValueError(kind)
    return bufs["S"].copy()


# ------------------------------------------------------------------ bass side

_CUSTOM_OPS = {}


def _register_op(name, spec):
    from concourse.dve_uop import DveOpSpec
    from concourse.dve_spec import lower
    from concourse import dve_ops
    from concourse.dve_ops import DveOp, OPS

    for op in OPS:
        if op.name == name:
            return op
    row = dve_ops._CUSTOM_DVE_ROW_BASE + len(OPS)
    shas = {}
    for ver in ("v3", "v4"):
        shas[ver] = DveOpSpec(
            name=name, opcode=row, uops=lower(spec, ver=ver), rd1_en=True
        ).sha(ver)
    op = DveOp(name, spec, subdim=False, uops_sha=shas)
    OPS.append(op)
    dve_ops._SUB_OPCODE_FOR_NAME[name] = row
    dve_ops.CUSTOM_DVE_SPECS[name] = spec
    return op


def _get_custom_ops():
    """Register fused DVE ops (idempotent): ROT2 out = s0*in0 + s1*in1,
    SQSUM out = in0^2 + in1^2."""
    if _CUSTOM_OPS:
        return _CUSTOM_OPS
    from concourse.dve_spec import Spec, Src0, Src1, C0, C1, sq

    _CUSTOM_OPS["rot2"] = _register_op(
        "ROT2_ANT",
        Spec(
            body=Src0 * C0 + Src1 * C1,
            reference=lambda in0, in1, s0, s1, imm2: (
                np.asarray(in0, np.float32) * np.asarray(s0, np.float32)
                + np.asarray(in1, np.float32) * np.asarray(s1, np.float32)
            ).astype(np.float32),
        ),
    )
    _CUSTOM_OPS["sqsum"] = _register_op(
        "SQSUM_ANT",
        Spec(
            body=sq(Src0) + sq(Src1),
            reference=lambda in0, in1, s0, s1, imm2: (
                np.asarray(in0, np.float32) ** 2 + np.asarray(in1, np.float32) ** 2
            ).astype(np.float32),
        ),
    )
    return _CUSTOM_OPS


def _ap(bass_mod, tile_ap, reg):
    t = tile_ap.tensor
    part = list(tile_ap.ap)[0]
    dims = [[part[0], part[1]]] + [[s, c] for s, c in reg[2]]
    return bass_mod.AP(t, tile_ap.offset + reg[1], dims)


def build_bass():
    import concourse.bass as bass
    import concourse.mybir as mybir
    import concourse.tile as tile
    from concourse import bacc
    from contextlib import ExitStack

    f32 = mybir.dt.float32
    f16 = mybir.dt.float16
    nc = bacc.Bacc("TRN2", target_bir_lowering=False, debug=False)
    ang_d = nc.dram_tensor("ang", [BPC, NCOLS], f32, kind="ExternalInput").ap()
    out_d = nc.dram_tensor("out", [BPC, 64], f32, kind="ExternalOutput").ap()

    sched = build_schedule()
    cops = _get_custom_ops()

    with tile.TileContext(nc) as tc, ExitStack() as ctx:
        state_p = ctx.enter_context(tc.tile_pool(name="state", bufs=1))
        io_p = ctx.enter_context(tc.tile_pool(name="io", bufs=2))

        A_t = state_p.tile([PT, F], f16, tag="A")
        B_t = state_p.tile([PT, F], f16, tag="B")
        T_t = state_p.tile([PT, NS], f16, tag="T")
        U_t = state_p.tile([PT, NS], f16, tag="U")
        for t in range(NTILES):
            ang_t = io_p.tile([PT, NCOLS], f32, tag="ang")
            s64_t = io_p.tile([PT, 64], f32, tag="s64")
            nc.sync.dma_start(ang_t[:], ang_d[t * PT : (t + 1) * PT, :])

            tiles = {"A": A_t[:], "B": B_t[:], "T": T_t[:], "U": U_t[:],
                     "S": s64_t[:]}
            nc.vector.memset(A_t[:], 0.0)
            nc.vector.memset(A_t[:, 0:1], 1.0)

            def scal(c):
                return ang_t[:, c : c + 1]

            def ap(reg):
                return _ap(bass, tiles[reg[0]], reg)

            for op in sched:
                kind = op[0]
                if kind == "rot2":
                    _, dst, s0, s1, c0, c1 = op
                    nc.vector._custom_dve(
                        cops["rot2"],
                        out=ap(dst), in0=ap(s0), in1=ap(s1),
                        s0=scal(c0), s1=scal(c1),
                    )
                elif kind == "cp":
                    _, dst, src = op
                    nc.scalar.mul(ap(dst), ap(src), 1.0)
                elif kind == "sqsum":
                    _, dst, s0, s1 = op
                    nc.vector._custom_dve(
                        cops["sqsum"], out=ap(dst), in0=ap(s0), in1=ap(s1)
                    )
                elif kind == "red":
                    _, dst, src = op
                    nc.vector.tensor_reduce(
                        ap(dst), ap(src),
                        axis=mybir.AxisListType.X,
                        op=mybir.AluOpType.add,
                    )
                else:
                    raise ValueError(kind)
            nc.sync.dma_start(out_d[t * PT : (t + 1) * PT, :], s64_t[:])
    nc.compile()
    return nc


_NC_CACHE = None


def run_cores(ang_full, trace=False, **kw):
    """ang_full: (B, NCOLS). Returns (B, 64) block sums + BassKernelResults."""
    global _NC_CACHE
    from concourse.bass_utils import run_bass_kernel_spmd

    if _NC_CACHE is None:
        _NC_CACHE = build_bass()
    nc = _NC_CACHE
    in_maps = [
        {"ang": np.ascontiguousarray(ang_full[c * BPC : (c + 1) * BPC])}
        for c in range(NCORES)
    ]
    last_err = None
    for attempt in range(3):
        try:
            res = run_bass_kernel_spmd(nc, in_maps, core_ids=list(range(NCORES)),
                                       trace=trace, **kw)
            break
        except Exception as e:  # device occasionally needs a cooldown
            last_err = e
            import time as _time

            _time.sleep(45 * (attempt + 1))
    else:
        raise last_err
    s64 = np.concatenate([r["out"] for r in res.results], axis=0)
    return s64, res


def kernel(x, input_scaling, weights, action_scale, action_bias):
    tab = coef_table(x, input_scaling, weights)
    s64, _ = run_cores(tab)
    return postprocess(s64, action_scale, action_bias)


# revision 9
# speedup vs baseline: 2.4145x; 2.4145x over previous
"""Trainium2 Bass kernel: 14-qubit data-reuploading quantum circuit actor.

Circuit per layer l (NL=5):
  for w in 0..13:  RY(in_scale[l,w]*x[:,w]) ; RZ(in_scale[l,w+14]*x[:,w]) on wire w
  for w in 0..13:  RZ(weights[l,w]) on wire w          (merged into input RZ)
  for w in 0..13:  RY(weights[l,w+14]) on wire w
  CNOT ring (i -> i+1 mod 14)
Then <Z_w> for w in 0..5, * action_scale + action_bias.

This environment's cost model (measured): per-instruction ~30-100us,
per-element ~0, per-call fixed ~200ms.  So the kernel minimizes
INSTRUCTION COUNT:

  - state: ONE fp16 SBUF plane per buffer, complex-interleaved
    [128 batch-partitions, 32768 floats] (float f = 2*amp + (0=re,1=im)),
    double-buffered A<->B: every gate reads A, writes B, swap.  No
    copy-backs, no temp shuffling, no deferred cosines (exact rotations
    keep |amp|<=1, fp16-safe).
  - custom DVE op ROT2: out = s0*in0 + s1*in1 (two per-partition scalars)
    -> a full RY gate (both complex comps, both halves) = 2 instructions;
    RZ = 4 (per-half phase sign differs); RY_weight with ring-CNOT folded
    into write APs = 4 (2 for wire 0).
  - CNOT(13,0) folded into next layer's RY(0) read APs; for the last
    layer folded into the sqsum (measurement) read APs.
  - measurement: SQSUM custom op (re^2+im^2) -> 64 block sums on chip;
    +/- signs applied on host.
  ~707 instructions per 128-row tile, 2 tiles per core (batch 2048 over
  8 cores = 256 rows/core).
"""

import os
import numpy as np

NQ = 14
NL = 5
OBS = 14
NA = 6
B = 2048
NCORES = 8
BPC = B // NCORES          # 256 batch rows per core
PT = 128                   # partitions (batch rows) per tile
NTILES = BPC // PT         # 2
NS = 1 << NQ               # 16384 amplitudes
F = 2 * NS                 # 32768 floats per row (complex-interleaved)
NCPG = 9                   # coef columns per (layer, wire)
NCOLS = NL * NQ * NCPG     # 630
# col offsets within a (l,w) group
RY_C, RY_S, RY_NS = 0, 1, 2
RZ_C, RZ_S, RZ_NS = 3, 4, 5
WY_C, WY_S, WY_NS = 6, 7, 8

# ---------------------------------------------------------------- host tables


def col(l, w, k):
    return (l * NQ + w) * NCPG + k


def coef_table(x, input_scaling, weights):
    """(n, NCOLS) f32 of cos/sin half-angle coefficients per (l, w)."""
    x = np.asarray(x, np.float64)
    isc = np.asarray(input_scaling, np.float64)
    wt = np.asarray(weights, np.float64)
    n = x.shape[0]
    tab = np.zeros((n, NL, NQ, NCPG), np.float64)
    for l in range(NL):
        for w in range(NQ):
            a_ry = isc[l, w] * x[:, w] / 2.0
            a_rz = (isc[l, w + OBS] * x[:, w] + wt[l, w]) / 2.0
            a_wy = np.full(n, wt[l, w + NQ] / 2.0)
            for base, a in ((RY_C, a_ry), (RZ_C, a_rz), (WY_C, a_wy)):
                tab[:, l, w, base] = np.cos(a)
                tab[:, l, w, base + 1] = np.sin(a)
                tab[:, l, w, base + 2] = -np.sin(a)
    return tab.reshape(n, NCOLS).astype(np.float32)


def postprocess(s64, action_scale, action_bias):
    """s64: (n, 64) block sums (blocks = top-6 amp bits). -> (n, NA) f32."""
    blk = np.arange(64)
    out = np.zeros((s64.shape[0], NA), np.float32)
    for w in range(NA):
        sign = 1.0 - 2.0 * ((blk >> (5 - w)) & 1)
        out[:, w] = s64 @ sign.astype(np.float32)
    return out * np.asarray(action_scale, np.float32) + np.asarray(
        action_bias, np.float32
    )


# ------------------------------------------------------------- gate schedule
# region = (buf, offset, dims); dims = tuple of (step, count), innermost
# last, in float-index space (f = 2*amp + comp).  Buffers: "A"/"B" full
# state planes, "T"/"U" 16384-float scratch (compacted wire halves; "T"
# doubles as the probability plane P at measurement), "S" the s64 output.
# ops:
#   ("rot2", dst, s0, s1, c0, c1): dst = c0*s0 + c1*s1.  The custom-DVE
#       TTSS encoding requires s1 to have a SINGLE free dim (rank-1).
#   ("cp", dst, src):              dst = src (staging copy)
#   ("sqsum", dst, s0, s1):        dst = s0^2 + s1^2 (rank-1 s1)
#   ("red", dst, src):             s64 block reduce


def _norm(dims):
    """drop unit dims, merge contiguous, assert rank<=2."""
    d = [(s, c) for s, c in dims if c != 1]
    out = []
    for s, c in d:
        if out and out[-1][0] == s * c:
            out[-1] = (s, c * out[-1][1])
        else:
            out.append((s, c))
    if not out:
        out = [(1, 1)]
    assert len(out) <= 2, out
    return tuple(out)


def _reg(buf, off, *dims):
    return (buf, off, _norm(dims))


class Sched:
    def __init__(self):
        self.ops = []
        self.cur = "A"

    def swap(self):
        self.cur = "B" if self.cur == "A" else "A"

    def rot2(self, dst, s0, s1, c0, c1):
        assert len(s1[2]) == 1, ("rot2 s1 must be rank-1", s1)
        self.ops.append(("rot2", dst, s0, s1, c0, c1))

    def cp(self, dst, src):
        self.ops.append(("cp", dst, src))

    # wire-0 gates: halves are contiguous -> rank-1, no staging ------------
    def ry0(self, cc, cs, cns):
        a, b = self.cur, "B" if self.cur == "A" else "A"
        X = _reg(a, 0, (1, NS))
        Y = _reg(a, NS, (1, NS))
        self.rot2(_reg(b, 0, (1, NS)), X, Y, cc, cns)
        self.rot2(_reg(b, NS, (1, NS)), X, Y, cs, cc)
        self.swap()

    def ry0_fold(self, cc, cs, cns):
        """RY(0) reading through the previous layer's CNOT(13, 0): stage the
        logical Y half into U piece-major (lsb0 | lsb1), then 4 rot2."""
        a, b = self.cur, "B" if self.cur == "A" else "A"
        d = ((4, 4096), (1, 2))
        E00, E01 = _reg(a, 0, *d), _reg(a, 2, *d)
        E10, E11 = _reg(a, 16384, *d), _reg(a, 16386, *d)
        U0, U1 = _reg("U", 0, (1, 8192)), _reg("U", 8192, (1, 8192))
        self.cp(U0, E10)  # logical Y, lsb=0
        self.cp(U1, E01)  # logical Y, lsb=1 (msb-flipped physically)
        B00, B01 = _reg(b, 0, *d), _reg(b, 2, *d)
        B10, B11 = _reg(b, 16384, *d), _reg(b, 16386, *d)
        self.rot2(B00, E00, U0, cc, cns)
        self.rot2(B10, E00, U0, cs, cc)
        self.rot2(B01, E11, U1, cc, cns)  # lsb=1: X lives msb-flipped
        self.rot2(B11, E11, U1, cs, cc)
        self.swap()

    def rz0(self, cc, cs, cns):
        a, b = self.cur, "B" if self.cur == "A" else "A"

        def r(buf, off):
            return _reg(buf, off, (2, 8192))

        self.rot2(r(b, 0), r(a, 0), r(a, 1), cc, cs)
        self.rot2(r(b, 1), r(a, 0), r(a, 1), cns, cc)
        self.rot2(r(b, NS), r(a, NS), r(a, NS + 1), cc, cns)
        self.rot2(r(b, NS + 1), r(a, NS), r(a, NS + 1), cs, cc)
        self.swap()

    # generic wires: RY into scratch T/U, RZ from scratch back to state ----
    def ry_rz(self, w, ryc, rys, ryns, rzc, rzs, rzns):
        a, b = self.cur, "B" if self.cur == "A" else "A"
        sa = 1 << (13 - w)
        nb = 1 << w
        X = _reg(a, 0, (4 * sa, nb), (1, 2 * sa))
        Y = _reg(a, 2 * sa, (4 * sa, nb), (1, 2 * sa))
        T = _reg("T", 0, (1, NS))
        U = _reg("U", 0, (1, NS))
        self.cp(U, Y)
        self.rot2(T, X, U, ryc, ryns)   # X' compacted
        self.rot2(U, X, U, rys, ryc)    # Y' in place
        Tre, Tim = _reg("T", 0, (2, 8192)), _reg("T", 1, (2, 8192))
        Ure, Uim = _reg("U", 0, (2, 8192)), _reg("U", 1, (2, 8192))
        Xre = _reg(b, 0, (4 * sa, nb), (2, sa))
        Xim = _reg(b, 1, (4 * sa, nb), (2, sa))
        Yre = _reg(b, 2 * sa, (4 * sa, nb), (2, sa))
        Yim = _reg(b, 2 * sa + 1, (4 * sa, nb), (2, sa))
        self.rot2(Xre, Tre, Tim, rzc, rzs)
        self.rot2(Xim, Tre, Tim, rzns, rzc)
        self.rot2(Yre, Ure, Uim, rzc, rzns)
        self.rot2(Yim, Ure, Uim, rzs, rzc)
        self.swap()

    def ry_fold_cnot(self, t, cc, cs, cns):
        """RY_w(t) with CNOT(t-1, t) folded into write APs (t >= 1)."""
        a, b = self.cur, "B" if self.cur == "A" else "A"
        fst = 2 * (1 << (13 - t))
        pt = 2 * fst
        P2 = 2 * pt
        nb = 1 << (t - 1)

        def R(buf, ai, bi):
            return _reg(buf, ai * pt + bi * fst, (P2, nb), (1, fst))

        if t == 1:  # nb == 1: all regions rank-1, no staging
            s01, s11 = R(a, 0, 1), R(a, 1, 1)
        else:
            s01, s11 = _reg("T", 0, (1, 8192)), _reg("T", 8192, (1, 8192))
            self.cp(s01, R(a, 0, 1))
            self.cp(s11, R(a, 1, 1))
        self.rot2(R(b, 0, 0), R(a, 0, 0), s01, cc, cns)
        self.rot2(R(b, 0, 1), R(a, 0, 0), s01, cs, cc)
        self.rot2(R(b, 1, 1), R(a, 1, 0), s11, cc, cns)  # X' -> flipped
        self.rot2(R(b, 1, 0), R(a, 1, 0), s11, cs, cc)   # Y' -> flipped
        self.swap()

    def measurement(self):
        a = self.cur
        # P[amp] = |state[pi(amp)]|^2, pi = last layer's CNOT(13,0) perm.
        # P lives in T.
        self.ops.append(
            ("sqsum", _reg("T", 0, (2, 8192)),
             _reg(a, 0, (4, 8192)), _reg(a, 1, (4, 8192)))
        )
        self.ops.append(
            ("sqsum", _reg("T", 1, (2, 4096)),
             _reg(a, 16386, (4, 4096)), _reg(a, 16387, (4, 4096)))
        )
        self.ops.append(
            ("sqsum", _reg("T", 8193, (2, 4096)),
             _reg(a, 2, (4, 4096)), _reg(a, 3, (4, 4096)))
        )
        self.ops.append(
            ("red", ("S", 0, ((1, 64),)), ("T", 0, ((256, 64), (1, 256))))
        )


def build_schedule():
    S = Sched()
    for l in range(NL):
        for w in range(NQ):
            if w == 0:
                if l == 0:
                    S.ry0(col(l, 0, RY_C), col(l, 0, RY_S), col(l, 0, RY_NS))
                else:
                    S.ry0_fold(col(l, 0, RY_C), col(l, 0, RY_S), col(l, 0, RY_NS))
                S.rz0(col(l, 0, RZ_C), col(l, 0, RZ_S), col(l, 0, RZ_NS))
            else:
                S.ry_rz(
                    w,
                    col(l, w, RY_C), col(l, w, RY_S), col(l, w, RY_NS),
                    col(l, w, RZ_C), col(l, w, RZ_S), col(l, w, RZ_NS),
                )
        S.ry0(col(l, 0, WY_C), col(l, 0, WY_S), col(l, 0, WY_NS))
        for t in range(1, NQ):
            S.ry_fold_cnot(t, col(l, t, WY_C), col(l, t, WY_S), col(l, t, WY_NS))
    S.measurement()
    return S.ops


# ------------------------------------------------------------ numpy executor


def _indices(reg):
    _, off, dims = reg
    idx = np.array([0], np.int64)
    for st, ct in dims:
        idx = (idx[:, None] + (np.arange(ct, dtype=np.int64) * st)[None, :]).ravel()
    return off + idx


def simulate_numpy(tab, fp16=True):
    """tab: (n, NCOLS) f32 coef table -> (n, 64) block sums."""
    n = tab.shape[0]
    sdt = np.float16 if fp16 else np.float32
    bufs = {
        "A": np.zeros((n, F), sdt),
        "B": np.zeros((n, F), sdt),
        "T": np.zeros((n, NS), sdt),
        "U": np.zeros((n, NS), sdt),
        "S": np.zeros((n, 64), np.float32),
    }
    bufs["A"][:, 0] = 1.0
    A = tab
    for op in build_schedule():
        kind = op[0]
        if kind == "rot2":
            _, dst, s0, s1, c0, c1 = op
            v = (
                A[:, c0 : c0 + 1].astype(np.float32)
                * bufs[s0[0]][:, _indices(s0)].astype(np.float32)
                + A[:, c1 : c1 + 1].astype(np.float32)
                * bufs[s1[0]][:, _indices(s1)].astype(np.float32)
            )
            bufs[dst[0]][:, _indices(dst)] = v.astype(sdt)
        elif kind == "cp":
            _, dst, src = op
            bufs[dst[0]][:, _indices(dst)] = bufs[src[0]][:, _indices(src)]
        elif kind == "sqsum":
            _, dst, s0, s1 = op
            v = (
                bufs[s0[0]][:, _indices(s0)].astype(np.float32) ** 2
                + bufs[s1[0]][:, _indices(s1)].astype(np.float32) ** 2
            )
            bufs[dst[0]][:, _indices(dst)] = v.astype(sdt)
        elif kind == "red":
            _, dst, src = op
            v = bufs[src[0]][:, _indices(src)].astype(np.float32)
            bufs["S"][:, _indices(dst)] = v.reshape(n, 64, 256).sum(axis=2)
        else:
            raise ValueError(kind)
    return bufs["S"].copy()


# ------------------------------------------------------------------ bass side

_CUSTOM_OPS = {}


def _register_op(name, spec):
    from concourse.dve_uop import DveOpSpec
    from concourse.dve_spec import lower
    from concourse import dve_ops
    from concourse.dve_ops import DveOp, OPS

    for op in OPS:
        if op.name == name:
            return op
    row = dve_ops._CUSTOM_DVE_ROW_BASE + len(OPS)
    shas = {}
    for ver in ("v3", "v4"):
        shas[ver] = DveOpSpec(
            name=name, opcode=row, uops=lower(spec, ver=ver), rd1_en=True
        ).sha(ver)
    op = DveOp(name, spec, subdim=False, uops_sha=shas)
    OPS.append(op)
    dve_ops._SUB_OPCODE_FOR_NAME[name] = row
    dve_ops.CUSTOM_DVE_SPECS[name] = spec
    return op


def _get_custom_ops():
    """Register fused DVE ops (idempotent): ROT2 out = s0*in0 + s1*in1,
    SQSUM out = in0^2 + in1^2."""
    if _CUSTOM_OPS:
        return _CUSTOM_OPS
    from concourse.dve_spec import Spec, Src0, Src1, C0, C1, sq

    _CUSTOM_OPS["rot2"] = _register_op(
        "ROT2_ANT",
        Spec(
            body=Src0 * C0 + Src1 * C1,
            reference=lambda in0, in1, s0, s1, imm2: (
                np.asarray(in0, np.float32) * np.asarray(s0, np.float32)
                + np.asarray(in1, np.float32) * np.asarray(s1, np.float32)
            ).astype(np.float32),
        ),
    )
    _CUSTOM_OPS["sqsum"] = _register_op(
        "SQSUM_ANT",
        Spec(
            body=sq(Src0) + sq(Src1),
            reference=lambda in0, in1, s0, s1, imm2: (
                np.asarray(in0, np.float32) ** 2 + np.asarray(in1, np.float32) ** 2
            ).astype(np.float32),
        ),
    )
    return _CUSTOM_OPS


def _ap(bass_mod, tile_ap, reg):
    t = tile_ap.tensor
    part = list(tile_ap.ap)[0]
    dims = [[part[0], part[1]]] + [[s, c] for s, c in reg[2]]
    return bass_mod.AP(t, tile_ap.offset + reg[1], dims)


def build_bass():
    import concourse.bass as bass
    import concourse.mybir as mybir
    import concourse.tile as tile
    from concourse import bacc
    from contextlib import ExitStack

    f32 = mybir.dt.float32
    f16 = mybir.dt.float16
    nc = bacc.Bacc("TRN2", target_bir_lowering=False, debug=False)
    ang_d = nc.dram_tensor("ang", [BPC, NCOLS], f32, kind="ExternalInput").ap()
    out_d = nc.dram_tensor("out", [BPC, 64], f32, kind="ExternalOutput").ap()

    sched = build_schedule()
    cops = _get_custom_ops()

    with tile.TileContext(nc) as tc, ExitStack() as ctx:
        state_p = ctx.enter_context(tc.tile_pool(name="state", bufs=1))
        io_p = ctx.enter_context(tc.tile_pool(name="io", bufs=2))

        A_t = state_p.tile([PT, F], f16, tag="A")
        B_t = state_p.tile([PT, F], f16, tag="B")
        T_t = state_p.tile([PT, NS], f16, tag="T")
        U_t = state_p.tile([PT, NS], f16, tag="U")
        for t in range(NTILES):
            ang_t = io_p.tile([PT, NCOLS], f32, tag="ang")
            s64_t = io_p.tile([PT, 64], f32, tag="s64")
            nc.sync.dma_start(ang_t[:], ang_d[t * PT : (t + 1) * PT, :])

            tiles = {"A": A_t[:], "B": B_t[:], "T": T_t[:], "U": U_t[:],
                     "S": s64_t[:]}
            nc.vector.memset(A_t[:], 0.0)
            nc.vector.memset(A_t[:, 0:1], 1.0)

            def scal(c):
                return ang_t[:, c : c + 1]

            def ap(reg):
                return _ap(bass, tiles[reg[0]], reg)

            for op in sched:
                kind = op[0]
                if kind == "rot2":
                    _, dst, s0, s1, c0, c1 = op
                    nc.vector._custom_dve(
                        cops["rot2"],
                        out=ap(dst), in0=ap(s0), in1=ap(s1),
                        s0=scal(c0), s1=scal(c1),
                    )
                elif kind == "cp":
                    _, dst, src = op
                    if os.environ.get("QK_CP", "rot2") == "rot2":
                        # copy as rot2 with immediate scalars (cheaper q than
                        # scalar.mul here); in1 = finite junk, scaled by 0
                        n = 1
                        for _, c in dst[2]:
                            n *= c
                        junk = (src[0], 0, ((1, n),))
                        nc.vector._custom_dve(
                            cops["rot2"],
                            out=ap(dst), in0=ap(src), in1=ap(junk),
                            s0=1.0, s1=0.0,
                        )
                    else:
                        nc.scalar.mul(ap(dst), ap(src), 1.0)
                elif kind == "sqsum":
                    _, dst, s0, s1 = op
                    nc.vector._custom_dve(
                        cops["sqsum"], out=ap(dst), in0=ap(s0), in1=ap(s1)
                    )
                elif kind == "red":
                    _, dst, src = op
                    nc.vector.tensor_reduce(
                        ap(dst), ap(src),
                        axis=mybir.AxisListType.X,
                        op=mybir.AluOpType.add,
                    )
                else:
                    raise ValueError(kind)
            nc.sync.dma_start(out_d[t * PT : (t + 1) * PT, :], s64_t[:])
    nc.compile()
    return nc


_NC_CACHE = None
_RUNNER = None


class _Result:
    exec_time_ns = None


class Runner:
    """Persistent jitted SPMD executor: same lowering as
    bass_utils.run_bass_kernel_spmd's axon path (bass2jax.run_bass_via_pjrt)
    but the jax.jit(shard_map(...)) closure is built once and cached, so
    warm calls skip retracing (~165ms/call)."""

    def __init__(self, nc, n_cores=NCORES):
        import jax
        from jax.sharding import Mesh, PartitionSpec
        from jax.experimental.shard_map import shard_map
        from concourse import bass2jax
        import concourse.mybir as mybir

        bass2jax.install_neuronx_cc_hook()
        self.nc = nc
        self.n_cores = n_cores
        part_name = nc.partition_id_tensor.name if nc.partition_id_tensor else None
        in_names, out_names, out_avals, self.zero_shapes = [], [], [], []
        for alloc in nc.m.functions[0].allocations:
            if not isinstance(alloc, mybir.MemoryLocationSet):
                continue
            name = alloc.memorylocations[0].name
            if alloc.kind == "ExternalInput":
                if name != part_name:
                    in_names.append(name)
            elif alloc.kind == "ExternalOutput":
                out_names.append(name)
                shape = tuple(alloc.tensor_shape)
                dtype = mybir.dt.np(alloc.dtype)
                out_avals.append(jax.core.ShapedArray(shape, dtype))
                self.zero_shapes.append((shape, dtype))
        self.in_names = list(in_names)
        self.out_names = list(out_names)
        self.out_avals = out_avals
        n_params = len(in_names)
        n_outs = len(out_names)
        all_in = list(in_names) + list(out_names)
        if part_name is not None:
            all_in.append(part_name)
        donate = tuple(range(n_params, n_params + n_outs))

        def _body(*args):
            operands = list(args)
            if nc.partition_id_tensor is not None:
                operands.append(bass2jax.partition_id_tensor())
            outs = bass2jax._bass_exec_p.bind(
                *operands,
                out_avals=tuple(out_avals),
                in_names=tuple(all_in),
                out_names=tuple(out_names),
                lowering_input_output_aliases=(),
                sim_require_finite=True,
                sim_require_nnan=True,
                nc=nc,
            )
            return tuple(outs)

        devices = jax.devices()[:n_cores]
        mesh = Mesh(np.asarray(devices), ("core",))
        in_specs = (PartitionSpec("core"),) * (n_params + n_outs)
        out_specs = (PartitionSpec("core"),) * n_outs
        self.fn = jax.jit(
            shard_map(_body, mesh=mesh, in_specs=in_specs,
                      out_specs=out_specs, check_rep=False),
            donate_argnums=donate,
            keep_unused=True,
        )

    def __call__(self, in_maps):
        concat_in = [
            np.concatenate([m[name] for m in in_maps], axis=0)
            for name in self.in_names
        ]
        zeros = [
            np.zeros((self.n_cores * s[0], *s[1:]), d)
            for s, d in self.zero_shapes
        ]
        out_arrs = self.fn(*concat_in, *zeros)
        n = self.n_cores
        return [
            {
                name: np.asarray(out_arrs[i]).reshape(n, *self.out_avals[i].shape)[c]
                for i, name in enumerate(self.out_names)
            }
            for c in range(n)
        ]


def run_cores(ang_full, trace=False, **kw):
    """ang_full: (B, NCOLS). Returns (B, 64) block sums + result handle."""
    global _NC_CACHE, _RUNNER
    if _NC_CACHE is None:
        _NC_CACHE = build_bass()
    nc = _NC_CACHE
    in_maps = [
        {"ang": np.ascontiguousarray(ang_full[c * BPC : (c + 1) * BPC])}
        for c in range(NCORES)
    ]
    last_err = None
    for attempt in range(3):
        try:
            if _RUNNER is None:
                _RUNNER = Runner(nc)
            results = _RUNNER(in_maps)
            break
        except Exception as e:  # device occasionally needs a cooldown
            last_err = e
            import time as _time

            _time.sleep(45 * (attempt + 1))
    else:
        raise last_err
    s64 = np.concatenate([r["out"] for r in results], axis=0)
    return s64, _Result()


def kernel(x, input_scaling, weights, action_scale, action_bias):
    tab = coef_table(x, input_scaling, weights)
    s64, _ = run_cores(tab)
    return postprocess(s64, action_scale, action_bias)


# revision 16
# speedup vs baseline: 4.4034x; 1.8237x over previous
"""Trainium2 Bass kernel: 14-qubit data-reuploading quantum circuit actor.

Circuit per layer l (NL=5):
  for w in 0..13:  RY(in_scale[l,w]*x[:,w]) ; RZ(in_scale[l,w+14]*x[:,w]) on wire w
  for w in 0..13:  RZ(weights[l,w]) on wire w          (merged into input RZ)
  for w in 0..13:  RY(weights[l,w+14]) on wire w
  CNOT ring (i -> i+1 mod 14)
Then <Z_w> for w in 0..5, * action_scale + action_bias.

This environment's cost model (measured): per-instruction ~30-100us,
per-element ~0, per-call fixed ~200ms.  So the kernel minimizes
INSTRUCTION COUNT:

  - state: ONE fp16 SBUF plane per buffer, complex-interleaved
    [128 batch-partitions, 32768 floats] (float f = 2*amp + (0=re,1=im)),
    double-buffered A<->B: every gate reads A, writes B, swap.  No
    copy-backs, no temp shuffling, no deferred cosines (exact rotations
    keep |amp|<=1, fp16-safe).
  - custom DVE op ROT2: out = s0*in0 + s1*in1 (two per-partition scalars)
    -> a full RY gate (both complex comps, both halves) = 2 instructions;
    RZ = 4 (per-half phase sign differs); RY_weight with ring-CNOT folded
    into write APs = 4 (2 for wire 0).
  - CNOT(13,0) folded into next layer's RY(0) read APs; for the last
    layer folded into the sqsum (measurement) read APs.
  - measurement: SQSUM custom op (re^2+im^2) -> 64 block sums on chip;
    +/- signs applied on host.
  ~707 instructions per 128-row tile, 2 tiles per core (batch 2048 over
  8 cores = 256 rows/core).
"""

import os
import numpy as np

NQ = 14
NL = 5
OBS = 14
NA = 6
B = 2048
NCORES = 8
BPC = B // NCORES          # 256 batch rows per core
PT = 128                   # partitions (batch rows) per tile
NTILES = BPC // PT         # 2
NS = 1 << NQ               # 16384 amplitudes
F = 2 * NS                 # 32768 floats per row (complex-interleaved)
NANG = 3 * NL * NQ         # 210 half-angles per row: k = type*70 + l*14 + w
NCOLS = 3 * NANG           # 630 coef columns on-chip: [cos | sin | -sin]
# col(l, w, t): t in 0..8, type = t//3 (ry/rz/wy), kind = t%3 (c/s/ns)
RY_C, RY_S, RY_NS = 0, 1, 2
RZ_C, RZ_S, RZ_NS = 3, 4, 5
WY_C, WY_S, WY_NS = 6, 7, 8

# ---------------------------------------------------------------- host tables


def col(l, w, t):
    return (t % 3) * NANG + (t // 3) * (NL * NQ) + l * NQ + w


def a_table(x, input_scaling, weights):
    """(n, NANG) f32 of half-angles, k = type*70 + l*14 + w."""
    x = np.asarray(x, np.float64)
    isc = np.asarray(input_scaling, np.float64)
    wt = np.asarray(weights, np.float64)
    n = x.shape[0]
    tab = np.zeros((n, 3, NL, NQ), np.float64)
    for l in range(NL):
        for w in range(NQ):
            tab[:, 0, l, w] = isc[l, w] * x[:, w] / 2.0
            tab[:, 1, l, w] = (isc[l, w + OBS] * x[:, w] + wt[l, w]) / 2.0
            tab[:, 2, l, w] = wt[l, w + NQ] / 2.0
    return tab.reshape(n, NANG).astype(np.float32)


def coef_table(a):
    """(n, NCOLS) f32 [cos | sin | -sin] of the half-angle table (sim only;
    on device this is computed by wrap + Sin activations)."""
    a = np.asarray(a, np.float64)
    return np.concatenate(
        [np.cos(a), np.sin(a), -np.sin(a)], axis=1
    ).astype(np.float32)


def postprocess(s64, action_scale, action_bias):
    """s64: (n, 64) block sums (blocks = top-6 amp bits). -> (n, NA) f32.
    (Numpy-sim path; on device the sign contraction runs on-chip.)"""
    blk = np.arange(64)
    out = np.zeros((s64.shape[0], NA), np.float32)
    for w in range(NA):
        sign = 1.0 - 2.0 * ((blk >> (5 - w)) & 1)
        out[:, w] = s64 @ sign.astype(np.float32)
    return out * np.asarray(action_scale, np.float32) + np.asarray(
        action_bias, np.float32
    )


# ------------------------------------------------------------- gate schedule
# region = (buf, offset, dims); dims = tuple of (step, count), innermost
# last, in float-index space (f = 2*amp + comp).  Buffers: "A"/"B" full
# state planes, "T"/"U" 16384-float scratch (compacted wire halves; "T"
# doubles as the probability plane P at measurement), "S" the s64 output.
# ops:
#   ("rot2", dst, s0, s1, c0, c1): dst = c0*s0 + c1*s1.  The custom-DVE
#       TTSS encoding requires s1 to have a SINGLE free dim (rank-1).
#   ("cp", dst, src):              dst = src (staging copy)
#   ("sqsum", dst, s0, s1):        dst = s0^2 + s1^2 (rank-1 s1)
#   ("red", dst, src):             s64 block reduce


def _norm(dims):
    """drop unit dims, merge contiguous, assert rank<=2."""
    d = [(s, c) for s, c in dims if c != 1]
    out = []
    for s, c in d:
        if out and out[-1][0] == s * c:
            out[-1] = (s, c * out[-1][1])
        else:
            out.append((s, c))
    if not out:
        out = [(1, 1)]
    assert len(out) <= 2, out
    return tuple(out)


def _reg(buf, off, *dims):
    return (buf, off, _norm(dims))


class Sched:
    def __init__(self):
        self.ops = []
        self.cur = "A"

    def swap(self):
        self.cur = "B" if self.cur == "A" else "A"

    def rot2(self, dst, s0, s1, c0, c1):
        assert len(s1[2]) == 1, ("rot2 s1 must be rank-1", s1)
        self.ops.append(("rot2", dst, s0, s1, c0, c1))

    def cp(self, dst, src):
        self.ops.append(("cp", dst, src))

    # wire-0 gates: halves are contiguous -> rank-1, no staging ------------
    def ry0(self, cc, cs, cns):
        a, b = self.cur, "B" if self.cur == "A" else "A"
        X = _reg(a, 0, (1, NS))
        Y = _reg(a, NS, (1, NS))
        self.rot2(_reg(b, 0, (1, NS)), X, Y, cc, cns)
        self.rot2(_reg(b, NS, (1, NS)), X, Y, cs, cc)
        self.swap()

    def ry0_fold(self, cc, cs, cns):
        """RY(0) reading through the previous layer's CNOT(13, 0): stage the
        logical Y half into U piece-major (lsb0 | lsb1), then 4 rot2."""
        a, b = self.cur, "B" if self.cur == "A" else "A"
        d = ((4, 4096), (1, 2))
        E00, E01 = _reg(a, 0, *d), _reg(a, 2, *d)
        E10, E11 = _reg(a, 16384, *d), _reg(a, 16386, *d)
        U0, U1 = _reg("U", 0, (1, 8192)), _reg("U", 8192, (1, 8192))
        self.cp(U0, E10)  # logical Y, lsb=0
        self.cp(U1, E01)  # logical Y, lsb=1 (msb-flipped physically)
        B00, B01 = _reg(b, 0, *d), _reg(b, 2, *d)
        B10, B11 = _reg(b, 16384, *d), _reg(b, 16386, *d)
        self.rot2(B00, E00, U0, cc, cns)
        self.rot2(B10, E00, U0, cs, cc)
        self.rot2(B01, E11, U1, cc, cns)  # lsb=1: X lives msb-flipped
        self.rot2(B11, E11, U1, cs, cc)
        self.swap()

    def rz0(self, cc, cs, cns):
        a, b = self.cur, "B" if self.cur == "A" else "A"

        def r(buf, off):
            return _reg(buf, off, (2, 8192))

        self.rot2(r(b, 0), r(a, 0), r(a, 1), cc, cs)
        self.rot2(r(b, 1), r(a, 0), r(a, 1), cns, cc)
        self.rot2(r(b, NS), r(a, NS), r(a, NS + 1), cc, cns)
        self.rot2(r(b, NS + 1), r(a, NS), r(a, NS + 1), cs, cc)
        self.swap()

    # generic wires: RY into scratch T/U, RZ from scratch back to state ----
    def ry_rz(self, w, ryc, rys, ryns, rzc, rzs, rzns):
        a, b = self.cur, "B" if self.cur == "A" else "A"
        sa = 1 << (13 - w)
        nb = 1 << w
        X = _reg(a, 0, (4 * sa, nb), (1, 2 * sa))
        Y = _reg(a, 2 * sa, (4 * sa, nb), (1, 2 * sa))
        T = _reg("T", 0, (1, NS))
        U = _reg("U", 0, (1, NS))
        self.cp(U, Y)
        self.rot2(T, X, U, ryc, ryns)   # X' compacted
        self.rot2(U, X, U, rys, ryc)    # Y' in place
        Tre, Tim = _reg("T", 0, (2, 8192)), _reg("T", 1, (2, 8192))
        Ure, Uim = _reg("U", 0, (2, 8192)), _reg("U", 1, (2, 8192))
        Xre = _reg(b, 0, (4 * sa, nb), (2, sa))
        Xim = _reg(b, 1, (4 * sa, nb), (2, sa))
        Yre = _reg(b, 2 * sa, (4 * sa, nb), (2, sa))
        Yim = _reg(b, 2 * sa + 1, (4 * sa, nb), (2, sa))
        self.rot2(Xre, Tre, Tim, rzc, rzs)
        self.rot2(Xim, Tre, Tim, rzns, rzc)
        self.rot2(Yre, Ure, Uim, rzc, rzns)
        self.rot2(Yim, Ure, Uim, rzs, rzc)
        self.swap()

    def ry_fold_cnot(self, t, cc, cs, cns):
        """RY_w(t) with CNOT(t-1, t) folded into write APs (t >= 1)."""
        a, b = self.cur, "B" if self.cur == "A" else "A"
        fst = 2 * (1 << (13 - t))
        pt = 2 * fst
        P2 = 2 * pt
        nb = 1 << (t - 1)

        def R(buf, ai, bi):
            return _reg(buf, ai * pt + bi * fst, (P2, nb), (1, fst))

        if t == 1:  # nb == 1: all regions rank-1, no staging
            s01, s11 = R(a, 0, 1), R(a, 1, 1)
        else:
            s01, s11 = _reg("T", 0, (1, 8192)), _reg("T", 8192, (1, 8192))
            self.cp(s01, R(a, 0, 1))
            self.cp(s11, R(a, 1, 1))
        self.rot2(R(b, 0, 0), R(a, 0, 0), s01, cc, cns)
        self.rot2(R(b, 0, 1), R(a, 0, 0), s01, cs, cc)
        self.rot2(R(b, 1, 1), R(a, 1, 0), s11, cc, cns)  # X' -> flipped
        self.rot2(R(b, 1, 0), R(a, 1, 0), s11, cs, cc)   # Y' -> flipped
        self.swap()

    def measurement(self):
        a = self.cur
        # P[amp] = |state[pi(amp)]|^2, pi = last layer's CNOT(13,0) perm.
        # P lives in T.
        self.ops.append(
            ("sqsum", _reg("T", 0, (2, 8192)),
             _reg(a, 0, (4, 8192)), _reg(a, 1, (4, 8192)))
        )
        self.ops.append(
            ("sqsum", _reg("T", 1, (2, 4096)),
             _reg(a, 16386, (4, 4096)), _reg(a, 16387, (4, 4096)))
        )
        self.ops.append(
            ("sqsum", _reg("T", 8193, (2, 4096)),
             _reg(a, 2, (4, 4096)), _reg(a, 3, (4, 4096)))
        )
        self.ops.append(
            ("red", ("S", 0, ((1, 64),)), ("T", 0, ((256, 64), (1, 256))))
        )


def build_schedule():
    S = Sched()
    for l in range(NL):
        for w in range(NQ):
            if w == 0:
                if l == 0:
                    S.ry0(col(l, 0, RY_C), col(l, 0, RY_S), col(l, 0, RY_NS))
                else:
                    S.ry0_fold(col(l, 0, RY_C), col(l, 0, RY_S), col(l, 0, RY_NS))
                S.rz0(col(l, 0, RZ_C), col(l, 0, RZ_S), col(l, 0, RZ_NS))
            else:
                S.ry_rz(
                    w,
                    col(l, w, RY_C), col(l, w, RY_S), col(l, w, RY_NS),
                    col(l, w, RZ_C), col(l, w, RZ_S), col(l, w, RZ_NS),
                )
        S.ry0(col(l, 0, WY_C), col(l, 0, WY_S), col(l, 0, WY_NS))
        for t in range(1, NQ):
            S.ry_fold_cnot(t, col(l, t, WY_C), col(l, t, WY_S), col(l, t, WY_NS))
    S.measurement()
    return S.ops


# ------------------------------------------------------------ numpy executor


def _indices(reg):
    _, off, dims = reg
    idx = np.array([0], np.int64)
    for st, ct in dims:
        idx = (idx[:, None] + (np.arange(ct, dtype=np.int64) * st)[None, :]).ravel()
    return off + idx


def simulate_numpy(a, fp16=True):
    """a: (n, NANG) f32 half-angle table -> (n, 64) block sums."""
    tab = coef_table(a)
    n = tab.shape[0]
    sdt = np.float16 if fp16 else np.float32
    bufs = {
        "A": np.zeros((n, F), sdt),
        "B": np.zeros((n, F), sdt),
        "T": np.zeros((n, NS), sdt),
        "U": np.zeros((n, NS), sdt),
        "S": np.zeros((n, 64), np.float32),
    }
    bufs["A"][:, 0] = 1.0
    A = tab
    for op in build_schedule():
        kind = op[0]
        if kind == "rot2":
            _, dst, s0, s1, c0, c1 = op
            v = (
                A[:, c0 : c0 + 1].astype(np.float32)
                * bufs[s0[0]][:, _indices(s0)].astype(np.float32)
                + A[:, c1 : c1 + 1].astype(np.float32)
                * bufs[s1[0]][:, _indices(s1)].astype(np.float32)
            )
            bufs[dst[0]][:, _indices(dst)] = v.astype(sdt)
        elif kind == "cp":
            _, dst, src = op
            bufs[dst[0]][:, _indices(dst)] = bufs[src[0]][:, _indices(src)]
        elif kind == "sqsum":
            _, dst, s0, s1 = op
            v = (
                bufs[s0[0]][:, _indices(s0)].astype(np.float32) ** 2
                + bufs[s1[0]][:, _indices(s1)].astype(np.float32) ** 2
            )
            bufs[dst[0]][:, _indices(dst)] = v.astype(sdt)
        elif kind == "red":
            _, dst, src = op
            v = bufs[src[0]][:, _indices(src)].astype(np.float32)
            bufs["S"][:, _indices(dst)] = v.reshape(n, 64, 256).sum(axis=2)
        else:
            raise ValueError(kind)
    return bufs["S"].copy()


# ------------------------------------------------------------------ bass side

_CUSTOM_OPS = {}


def _register_op(name, spec):
    from concourse.dve_uop import DveOpSpec
    from concourse.dve_spec import lower
    from concourse import dve_ops
    from concourse.dve_ops import DveOp, OPS

    for op in OPS:
        if op.name == name:
            return op
    row = dve_ops._CUSTOM_DVE_ROW_BASE + len(OPS)
    shas = {}
    for ver in ("v3", "v4"):
        shas[ver] = DveOpSpec(
            name=name, opcode=row, uops=lower(spec, ver=ver), rd1_en=True
        ).sha(ver)
    op = DveOp(name, spec, subdim=False, uops_sha=shas)
    OPS.append(op)
    dve_ops._SUB_OPCODE_FOR_NAME[name] = row
    dve_ops.CUSTOM_DVE_SPECS[name] = spec
    return op


def _get_custom_ops():
    """Register fused DVE ops (idempotent): ROT2 out = s0*in0 + s1*in1,
    SQSUM out = in0^2 + in1^2."""
    if _CUSTOM_OPS:
        return _CUSTOM_OPS
    from concourse.dve_spec import Spec, Src0, Src1, C0, C1, sq

    _CUSTOM_OPS["rot2"] = _register_op(
        "ROT2_ANT",
        Spec(
            body=Src0 * C0 + Src1 * C1,
            reference=lambda in0, in1, s0, s1, imm2: (
                np.asarray(in0, np.float32) * np.asarray(s0, np.float32)
                + np.asarray(in1, np.float32) * np.asarray(s1, np.float32)
            ).astype(np.float32),
        ),
    )
    _CUSTOM_OPS["sqsum"] = _register_op(
        "SQSUM_ANT",
        Spec(
            body=sq(Src0) + sq(Src1),
            reference=lambda in0, in1, s0, s1, imm2: (
                np.asarray(in0, np.float32) ** 2 + np.asarray(in1, np.float32) ** 2
            ).astype(np.float32),
        ),
    )
    return _CUSTOM_OPS


def _ap(bass_mod, tile_ap, reg):
    t = tile_ap.tensor
    part = list(tile_ap.ap)[0]
    dims = [[part[0], part[1]]] + [[s, c] for s, c in reg[2]]
    return bass_mod.AP(t, tile_ap.offset + reg[1], dims)


def build_bass():
    import concourse.bass as bass
    import concourse.mybir as mybir
    import concourse.tile as tile
    from concourse import bacc
    from contextlib import ExitStack

    f32 = mybir.dt.float32
    f16 = mybir.dt.float16
    PI = float(np.pi)
    nc = bacc.Bacc("TRN2", target_bir_lowering=False, debug=False)
    a_d = nc.dram_tensor("a", [BPC, NANG], f32, kind="ExternalInput").ap()
    out_d = nc.dram_tensor("out", [BPC, NA], f32, kind="ExternalOutput").ap()

    sched = build_schedule()
    cops = _get_custom_ops()

    with tile.TileContext(nc) as tc, ExitStack() as ctx:
        state_p = ctx.enter_context(tc.tile_pool(name="state", bufs=1))
        io_p = ctx.enter_context(tc.tile_pool(name="io", bufs=2))

        A_t = state_p.tile([PT, F], f16, tag="A")
        B_t = state_p.tile([PT, F], f16, tag="B")
        T_t = state_p.tile([PT, NS], f16, tag="T")
        U_t = state_p.tile([PT, NS], f16, tag="U")
        W_t = state_p.tile([PT, 2 * NANG], f32, tag="W")   # wrapped angles
        sg_t = state_p.tile([PT, 6 * 64], f32, tag="sg")   # +-1 sign rows
        s64_t = state_p.tile([PT, 64], f32, tag="s64")
        r64_t = state_p.tile([PT, 64], f32, tag="r64")

        # sign rows for <Z_w>, w = 0..5: blocks of 2^(5-w) alternate +1/-1
        for w in range(6):
            r = 1 << (5 - w)
            nc.vector.memset(sg_t[:, w * 64 : (w + 1) * 64], 1.0)
            neg = bass.AP(
                sg_t[:].tensor,
                sg_t[:].offset + w * 64 + r,
                [list(sg_t[:].ap)[0], [2 * r, 32 // r], [1, r]],
            )
            nc.vector.memset(neg, -1.0)

        # minimax-ish polynomial sin/cos on [-pi, pi] (t = y^2):
        # sin(y) = y * sum P[k] t^(4-k);  cos(y) = sum Q[k] t^(5-k)
        SIN_P = [2.2248706406891887e-06, -0.00019424154210166545,
                 0.008319842398281522, -0.16665145941120196,
                 0.9999972898367918]
        COS_Q = [-2.219394993734796e-07, 2.42531924958235e-05,
                 -0.001386274731586208, 0.04166103279007339,
                 -0.4999955816555398, 0.9999994436793969]
        mul_op, add_op = mybir.AluOpType.mult, mybir.AluOpType.add
        for t in range(NTILES):
            a_t = io_p.tile([PT, NANG], f32, tag="a")
            ang_t = io_p.tile([PT, NCOLS], f32, tag="ang")
            out6_t = io_p.tile([PT, NA], f32, tag="out6")
            nc.sync.dma_start(a_t[:], a_d[t * PT : (t + 1) * PT, :])

            # coefs on-chip: wrap to [-pi, pi], then Horner in y^2
            y, t2 = W_t[:, 0:NANG], W_t[:, NANG : 2 * NANG]
            aC = ang_t[:, 0:NANG]
            aS = ang_t[:, NANG : 2 * NANG]
            aNS = ang_t[:, 2 * NANG : 3 * NANG]
            nc.vector.add_range_wrap(y, a_t[:], 0.0, PI, 2.0 * PI)
            nc.vector.tensor_mul(t2, y, y)
            nc.vector.tensor_scalar(aS, t2, SIN_P[0], SIN_P[1], mul_op, add_op)
            for ck in SIN_P[2:]:
                nc.vector.tensor_mul(aS, aS, t2)
                nc.vector.tensor_scalar_add(aS, aS, ck)
            nc.vector.tensor_mul(aS, aS, y)
            nc.vector.tensor_scalar(aC, t2, COS_Q[0], COS_Q[1], mul_op, add_op)
            for ck in COS_Q[2:]:
                nc.vector.tensor_mul(aC, aC, t2)
                nc.vector.tensor_scalar_add(aC, aC, ck)
            nc.vector.tensor_scalar_mul(aNS, aS, -1.0)

            tiles = {"A": A_t[:], "B": B_t[:], "T": T_t[:], "U": U_t[:],
                     "S": s64_t[:]}
            nc.vector.memset(A_t[:], 0.0)
            nc.vector.memset(A_t[:, 0:1], 1.0)

            def scal(c):
                return ang_t[:, c : c + 1]

            def ap(reg):
                return _ap(bass, tiles[reg[0]], reg)

            for op in sched:
                kind = op[0]
                if kind == "rot2":
                    _, dst, s0, s1, c0, c1 = op
                    nc.vector._custom_dve(
                        cops["rot2"],
                        out=ap(dst), in0=ap(s0), in1=ap(s1),
                        s0=scal(c0), s1=scal(c1),
                    )
                elif kind == "cp":
                    _, dst, src = op
                    if os.environ.get("QK_CP", "rot2") == "rot2":
                        # copy as rot2 with immediate scalars (cheaper q than
                        # scalar.mul here); in1 = finite junk, scaled by 0
                        n = 1
                        for _, c in dst[2]:
                            n *= c
                        junk = (src[0], 0, ((1, n),))
                        nc.vector._custom_dve(
                            cops["rot2"],
                            out=ap(dst), in0=ap(src), in1=ap(junk),
                            s0=1.0, s1=0.0,
                        )
                    else:
                        nc.scalar.mul(ap(dst), ap(src), 1.0)
                elif kind == "sqsum":
                    _, dst, s0, s1 = op
                    nc.vector._custom_dve(
                        cops["sqsum"], out=ap(dst), in0=ap(s0), in1=ap(s1)
                    )
                elif kind == "red":
                    _, dst, src = op
                    nc.vector.tensor_reduce(
                        ap(dst), ap(src),
                        axis=mybir.AxisListType.X,
                        op=mybir.AluOpType.add,
                    )
                else:
                    raise ValueError(kind)
            # on-chip sign contraction: out6[:, w] = sum_b s64[b]*sg[w, b]
            # (tensor_tensor_reduce is broken in this runtime - wedges the
            # device - so mul + reduce instead)
            for w in range(NA):
                nc.vector.tensor_mul(
                    r64_t[:], s64_t[:], sg_t[:, w * 64 : (w + 1) * 64]
                )
                nc.vector.tensor_reduce(
                    out6_t[:, w : w + 1], r64_t[:],
                    axis=mybir.AxisListType.X, op=mybir.AluOpType.add,
                )
            nc.sync.dma_start(out_d[t * PT : (t + 1) * PT, :], out6_t[:])
    nc.compile()
    return nc


_NC_CACHE = None
_RUNNER = None


class _Result:
    exec_time_ns = None


class Runner:
    """Persistent jitted SPMD executor: same lowering as
    bass_utils.run_bass_kernel_spmd's axon path (bass2jax.run_bass_via_pjrt)
    but the jax.jit(shard_map(...)) closure is built once and cached, so
    warm calls skip retracing (~165ms/call)."""

    def __init__(self, nc, n_cores=NCORES):
        import jax
        from jax.sharding import Mesh, PartitionSpec
        from jax.experimental.shard_map import shard_map
        from concourse import bass2jax
        import concourse.mybir as mybir

        bass2jax.install_neuronx_cc_hook()
        self.nc = nc
        self.n_cores = n_cores
        part_name = nc.partition_id_tensor.name if nc.partition_id_tensor else None
        in_names, out_names, out_avals, self.zero_shapes = [], [], [], []
        for alloc in nc.m.functions[0].allocations:
            if not isinstance(alloc, mybir.MemoryLocationSet):
                continue
            name = alloc.memorylocations[0].name
            if alloc.kind == "ExternalInput":
                if name != part_name:
                    in_names.append(name)
            elif alloc.kind == "ExternalOutput":
                out_names.append(name)
                shape = tuple(alloc.tensor_shape)
                dtype = mybir.dt.np(alloc.dtype)
                out_avals.append(jax.core.ShapedArray(shape, dtype))
                self.zero_shapes.append((shape, dtype))
        self.in_names = list(in_names)
        self.out_names = list(out_names)
        self.out_avals = out_avals
        n_params = len(in_names)
        n_outs = len(out_names)
        all_in = list(in_names) + list(out_names)
        if part_name is not None:
            all_in.append(part_name)
        donate = tuple(range(n_params, n_params + n_outs))

        def _body(*args):
            operands = list(args)
            if nc.partition_id_tensor is not None:
                operands.append(bass2jax.partition_id_tensor())
            outs = bass2jax._bass_exec_p.bind(
                *operands,
                out_avals=tuple(out_avals),
                in_names=tuple(all_in),
                out_names=tuple(out_names),
                lowering_input_output_aliases=(),
                sim_require_finite=True,
                sim_require_nnan=True,
                nc=nc,
            )
            return tuple(outs)

        devices = jax.devices()[:n_cores]
        mesh = Mesh(np.asarray(devices), ("core",))
        in_specs = (PartitionSpec("core"),) * (n_params + n_outs)
        out_specs = (PartitionSpec("core"),) * n_outs
        self.fn = jax.jit(
            shard_map(_body, mesh=mesh, in_specs=in_specs,
                      out_specs=out_specs, check_rep=False),
            donate_argnums=donate,
            keep_unused=True,
        )

    def __call__(self, in_maps):
        concat_in = [
            np.concatenate([m[name] for m in in_maps], axis=0)
            for name in self.in_names
        ]
        zeros = [
            np.zeros((self.n_cores * s[0], *s[1:]), d)
            for s, d in self.zero_shapes
        ]
        out_arrs = self.fn(*concat_in, *zeros)
        n = self.n_cores
        return [
            {
                name: np.asarray(out_arrs[i]).reshape(n, *self.out_avals[i].shape)[c]
                for i, name in enumerate(self.out_names)
            }
            for c in range(n)
        ]


def run_cores(a_full, trace=False, **kw):
    """a_full: (B, NANG) half-angles. Returns (B, NA) signed sums (no
    action scale/bias) + result handle."""
    global _NC_CACHE, _RUNNER
    if _NC_CACHE is None:
        _NC_CACHE = build_bass()
    nc = _NC_CACHE
    in_maps = [
        {"a": np.ascontiguousarray(a_full[c * BPC : (c + 1) * BPC])}
        for c in range(NCORES)
    ]
    last_err = None
    for attempt in range(3):
        try:
            if _RUNNER is None:
                _RUNNER = Runner(nc)
            results = _RUNNER(in_maps)
            break
        except Exception as e:  # device occasionally needs a cooldown
            last_err = e
            import time as _time

            _time.sleep(45 * (attempt + 1))
    else:
        raise last_err
    s6 = np.concatenate([r["out"] for r in results], axis=0)
    return s6, _Result()


def kernel(x, input_scaling, weights, action_scale, action_bias):
    a = a_table(x, input_scaling, weights)
    s6, _ = run_cores(a)
    return s6 * np.asarray(action_scale, np.float32) + np.asarray(
        action_bias, np.float32
    )
